# revision 42
# baseline (speedup 1.0000x reference)
"""Trainium2 Bass kernel for CachedMultiheadAttention (sliding-window + ALiBi).

Sharding: 8 cores = 2 batches x 4 head-quartets. Core c handles batch c//4 and
heads [4*(c%4), 4*(c%4)+4). Each core computes QKV projection for its heads,
banded attention (causal + 512 window + ALiBi), and a partial out-projection
over its heads' 256 embedding columns. Host sums the 4 partials per batch.

v2 design (all bf16 matmuls, PE kept saturated):
  - Q^T/K^T produced slot-by-slot so S-matmuls start 2/5 into the projection.
  - V produced in NATURAL layout straight from the projection (no PE
    transposes), with a 64-wide ones block per head: the AV matmul output
    rows 0-63 are AO^T and rows 64-127 are the softmax denominator Z
    replicated 64x -- the partition broadcast comes free from the matmul.
  - Normalize = reciprocal_approx_fast + one multiply (no serial
    reciprocal/partition_broadcast chain).
  - exp on ACT; band+ALiBi multiplicative bias on DVE/GpSimd alternating.
  - out-proj per 512-col group interleaved with the last AV groups.
"""
import math

import numpy as np
import ml_dtypes

import concourse.bass as bass
import concourse.tile as tile
from concourse import bacc, mybir
from concourse.bass_utils import run_bass_kernel_spmd

F32 = mybir.dt.float32
BF16 = mybir.dt.bfloat16

B, T, E, H, HD, W = 2, 2048, 1024, 16, 64, 512
NCORES = 8
HL = 4                # local heads per core
NT = T // 128         # 16 t-blocks

_CACHE = {}


def _get_slopes(n):
    def p2(m):
        start = 2 ** (-(2 ** (-(math.log2(m) - 3))))
        return [start * start**i for i in range(m)]
    if math.log2(n) % 1 == 0:
        return p2(n)
    c = 2 ** math.floor(math.log2(n))
    return p2(c) + _get_slopes(2 * c)[0::2][: n - c]


def _build(dbg=False):
    nc = bacc.Bacc("TRN2", target_bir_lowering=False, debug=False, num_devices=NCORES)
    # all inputs are partition-major SBUF images (>=2KB contiguous per
    # partition line; 1KB lines run the DMA engines at ~22GB/s)
    xT = nc.dram_tensor("xT", [128, 8, T], BF16, kind="ExternalInput").ap()
    wqk = nc.dram_tensor("wqk", [128, 4, 8, 128], BF16, kind="ExternalInput").ap()
    wvT = nc.dram_tensor("wvT", [128, 8, 256], BF16, kind="ExternalInput").ap()
    wo = nc.dram_tensor("wo", [128, 2, E], BF16, kind="ExternalInput").ap()
    biasd = nc.dram_tensor("biasd", [128, HL, 640], BF16, kind="ExternalInput").ap()
    outT = nc.dram_tensor("outT", [8, 128, T], BF16, kind="ExternalOutput").ap()
    if dbg:
        d_qkvT = nc.dram_tensor("d_qkvT", [128, 4, T], BF16, kind="ExternalOutput").ap()
        d_vnat = nc.dram_tensor("d_vnat", [128, NT, HL, 128], BF16, kind="ExternalOutput").ap()
        d_pth = nc.dram_tensor("d_pth", [128, NT, 640], BF16, kind="ExternalOutput").ap()
        d_ao2T = nc.dram_tensor("d_ao2T", [128, 2, T], BF16, kind="ExternalOutput").ap()

    with tile.TileContext(nc) as tc:
        with (
            tc.tile_pool(name="singles", bufs=1) as singles,
            tc.tile_pool(name="ptp", bufs=4) as ptp,
            tc.tile_pool(name="sprep", bufs=6) as sprep,
            tc.tile_pool(name="recp", bufs=3) as recp,
            tc.tile_pool(name="evp", bufs=3) as evp,
            tc.tile_pool(name="mm", bufs=3, space="PSUM") as mmp,
            tc.tile_pool(name="s1", bufs=2, space="PSUM") as s1p,
            tc.tile_pool(name="accp", bufs=3, space="PSUM") as accp,
        ):
            # --- one-time loads: dram layouts mirror the SBUF tiles, so
            # every transfer has >=2KB contiguous per-partition lines.
            # K0 slot chunk + x(ec, first half) gate the first matmul. ---
            wqk_sb = singles.tile([128, 4, 8, 128], BF16)
            for s in range(4):
                nc.gpsimd.dma_start(wqk_sb[:, s], wqk[:, s])
            xsb = singles.tile([128, 8, T], BF16)
            for th in range(2):
                for ec in range(8):
                    eng = nc.sync if ec % 2 == 0 else nc.scalar
                    eng.dma_start(
                        xsb[:, ec, th * 1024:(th + 1) * 1024],
                        xT[:, ec, th * 1024:(th + 1) * 1024])
            wvT_sb = singles.tile([128, 8, 256], BF16)
            nc.gpsimd.dma_start(wvT_sb[:], wvT)
            bias_sb = singles.tile([128, HL, 640], BF16)
            nc.gpsimd.dma_start(bias_sb[:], biasd)
            wo_sb = singles.tile([128, 2, E], BF16)
            nc.gpsimd.dma_start(wo_sb[:], wo)

            qkvT = singles.tile([128, 4, T], BF16)   # slots: K0 Q0 K1 Q1
            vnat = singles.tile([128, NT, HL, 128], BF16)
            nc.vector.memset(vnat[:, :, :, HD:128], 1.0)  # ones blocks only
            ao2T = singles.tile([128, 2, T], BF16)   # normalized AO^T

            # --- emit helpers ---
            def emit_qk_group(s, tb, copy_eng):
                pt = mmp.tile([128, 512], F32, tag="mm512")
                for ec in range(8):
                    nc.tensor.matmul(
                        pt[:],
                        lhsT=wqk_sb[:, s, ec, :],
                        rhs=xsb[:, ec, tb * 512:(tb + 1) * 512],
                        start=(ec == 0), stop=(ec == 7),
                    )
                if copy_eng is nc.scalar:
                    nc.scalar.copy(qkvT[:, s, tb * 512:(tb + 1) * 512], pt[:])
                else:
                    copy_eng.tensor_copy(
                        qkvT[:, s, tb * 512:(tb + 1) * 512], pt[:])

            def emit_v(tb16):
                pv = mmp.tile([128, 256], F32, tag="mm512")
                for ec in range(8):
                    nc.tensor.matmul(
                        pv[:],
                        lhsT=xsb[:, ec, tb16 * 128:(tb16 + 1) * 128],
                        rhs=wvT_sb[:, ec, :],
                        start=(ec == 0), stop=(ec == 7),
                    )
                nc.vector.tensor_copy(
                    vnat[:, tb16, :, 0:HD],
                    pv[:].rearrange("p (h d) -> p h d", h=HL))

            def emit_s(sq, pths, jb):
                ks, qs = 2 * sq, 2 * sq + 1
                qw = min(5, NT - jb) * 128
                w0 = min(qw, 512)
                for hh in range(2):
                    h = 2 * sq + hh
                    r0 = hh * 64
                    pth = pths[hh]
                    praw = sprep.tile([128, 640], BF16, tag="praw")
                    s5 = mmp.tile([128, 512], F32, tag="mm512")
                    nc.tensor.matmul(
                        s5[:, 0:w0],
                        lhsT=qkvT[r0:r0 + 64, ks, jb * 128:(jb + 1) * 128],
                        rhs=qkvT[r0:r0 + 64, qs, jb * 128:jb * 128 + w0],
                        start=True, stop=True,
                    )
                    nc.scalar.activation(
                        out=praw[:, 0:w0], in_=s5[:, 0:w0],
                        func=mybir.ActivationFunctionType.Exp,
                    )
                    if qw > 512:
                        s1 = s1p.tile([128, 128], F32, tag="s128")
                        nc.tensor.matmul(
                            s1[:],
                            lhsT=qkvT[r0:r0 + 64, ks, jb * 128:(jb + 1) * 128],
                            rhs=qkvT[r0:r0 + 64, qs, jb * 128 + 512:jb * 128 + qw],
                            start=True, stop=True,
                        )
                        nc.scalar.activation(
                            out=praw[:, 512:qw], in_=s1[:],
                            func=mybir.ActivationFunctionType.Exp,
                        )
                    # vector carries the PSUM-only AV recip/normalize chains,
                    # so bias mults spill to gpsimd (~1.8x slower) where
                    # vector is loaded: late phase-A and the rounds.
                    if sq == 0:
                        eng = nc.vector if (hh == 0 or jb < 8) else nc.gpsimd
                    else:
                        eng = nc.vector if (hh == 0 and jb % 2 == 0) else nc.gpsimd
                    eng.tensor_tensor(
                        out=pth[:, jb, 0:qw], in0=praw[:, 0:qw],
                        in1=bias_sb[:, h, 0:qw], op=mybir.AluOpType.mult,
                    )

            def emit_av_group(sq, pths, g):
                for hh in range(2):
                    h = 2 * sq + hh
                    r0 = hh * 64
                    pth = pths[hh]
                    ao = accp.tile([128, 512], F32, tag="acc")
                    jbs = [4 * g] + [jb for jb in range(max(0, 4 * g - 4), 4 * g + 4)
                                     if jb != 4 * g]
                    for i, jb in enumerate(jbs):
                        qb_lo = max(4 * g, jb)
                        qb_hi = min(4 * g + 3, jb + 4)
                        wdt = (qb_hi - qb_lo + 1) * 128
                        ao_off = (qb_lo - 4 * g) * 128
                        p_off = (qb_lo - jb) * 128
                        nc.tensor.matmul(
                            ao[:, ao_off:ao_off + wdt],
                            lhsT=vnat[:, jb, h, :],
                            rhs=pth[:, jb, p_off:p_off + wdt],
                            start=(i == 0), stop=(i == len(jbs) - 1),
                            skip_group_check=True,
                        )
                    # recip on the FULL tile (custom-DVE op needs base-0 full
                    # APs; rows 0:64 are garbage 1/AO values, never read).
                    # (gpsimd cannot touch PSUM; both stages live on vector)
                    rec = recp.tile([128, 512], F32, tag="rec")
                    nc.vector.reciprocal_approx_fast(out=rec[:], in_=ao[:])
                    nc.vector.tensor_tensor(
                        out=ao2T[r0:r0 + 64, sq, g * 512:(g + 1) * 512],
                        in0=ao[0:64, :], in1=rec[64:128, :],
                        op=mybir.AluOpType.mult,
                    )

            def emit_oproj(tb):
                for fc in range(8):
                    po = mmp.tile([128, 512], F32, tag="mm512")
                    for c2 in range(2):
                        nc.tensor.matmul(
                            po[:],
                            lhsT=wo_sb[:, c2, fc * 128:(fc + 1) * 128],
                            rhs=ao2T[:, c2, tb * 512:(tb + 1) * 512],
                            start=(c2 == 0), stop=(c2 == 1),
                        )
                    ev = evp.tile([128, 512], BF16, tag="ev")
                    if fc < 6:
                        nc.vector.tensor_copy(ev[:], po[:])
                    else:
                        nc.scalar.copy(ev[:], po[:])
                    nc.sync.dma_start(outT[fc, :, tb * 512:(tb + 1) * 512], ev[:])

            # --- schedule: emission follows x-chunk arrival so the PE is
            # dense from the first matmul (S0/V/K1Q1 fold into the stream;
            # exps start ~10us earlier than a phase-ordered emission) ---
            pth0a = ptp.tile([128, NT, 640], BF16, tag="pth")
            pth0b = ptp.tile([128, NT, 640], BF16, tag="pth")
            pths0 = [pth0a, pth0b]

            def qk(s, tb):
                # scalar queue is reserved for exps: a copy stuck behind a
                # 1us exp stalls the PE through the PSUM pool rotation
                emit_qk_group(s, tb, nc.vector)

            qk(0, 0); qk(1, 0)
            qk(0, 1); qk(1, 1)
            emit_s(0, pths0, 0); emit_s(0, pths0, 1); emit_v(0); emit_v(1)
            qk(0, 2); qk(1, 2)
            emit_s(0, pths0, 2); emit_s(0, pths0, 3); emit_v(2); emit_v(3)
            qk(2, 0); qk(3, 0)
            emit_s(0, pths0, 4); emit_s(0, pths0, 5); emit_v(4); emit_v(5)
            qk(0, 3); qk(1, 3)
            emit_s(0, pths0, 6); emit_s(0, pths0, 7); emit_v(6); emit_v(7)
            qk(2, 1); qk(3, 1)
            emit_s(0, pths0, 8); emit_s(0, pths0, 9); emit_v(8); emit_v(9)
            emit_av_group(0, pths0, 0)
            qk(2, 2); qk(3, 2)
            emit_s(0, pths0, 10); emit_s(0, pths0, 11); emit_v(10); emit_v(11)
            qk(2, 3); qk(3, 3)
            for jb in range(12, 16):
                emit_s(0, pths0, jb); emit_v(jb)
                if jb == 13:
                    emit_av_group(0, pths0, 1)
                if jb == 15:
                    emit_av_group(0, pths0, 2)
            pth1a = ptp.tile([128, NT, 640], BF16, tag="pth")
            pth1b = ptp.tile([128, NT, 640], BF16, tag="pth")
            pths1 = [pth1a, pth1b]
            for b in range(4):                    # rolling S1 / AV1 / oproj
                for jb in range(4 * b, 4 * b + 4):
                    emit_s(1, pths1, jb)
                if b == 0:
                    emit_av_group(0, pths0, 3)
                else:
                    emit_av_group(1, pths1, b - 1)
                    emit_oproj(b - 1)
            if dbg:
                nc.sync.dma_start(d_qkvT[:], qkvT[:])
                nc.sync.dma_start(d_vnat[:], vnat[:])
                nc.sync.dma_start(d_pth[:], pths0[0][:])
            emit_av_group(1, pths1, 3)
            emit_oproj(3)
            if dbg:
                nc.sync.dma_start(d_ao2T[:], ao2T[:])

    nc.compile()
    return nc


def _host_inputs(query, in_proj_weight, out_proj_weight):
    """Per-core input maps (numpy only)."""
    slopes = np.asarray(_get_slopes(H), np.float32)
    q32 = np.asarray(query, np.float32)
    w_in = np.asarray(in_proj_weight, np.float32)
    w_out = np.asarray(out_proj_weight, np.float32)

    jj = np.arange(128)[:, None]
    cc = np.arange(640)[None, :]
    allowed = (cc >= jj) & (cc - jj <= W)
    in_maps = []
    for c in range(NCORES):
        b, hq = divmod(c, 4)
        heads = np.arange(4 * hq, 4 * hq + HL)
        rows = (heads[:, None] * HD + np.arange(HD)[None, :]).reshape(-1)  # 256

        # slots K0 Q0 K1 Q1: slot s covers head pair s//2 (128 rows).
        # wqk image [p, s, ec, m] = W_slot_s[m, ec*128+p]
        wqk = np.empty((128, 4, 8, 128), ml_dtypes.bfloat16)
        for s in range(4):
            sq, is_q = s // 2, (s % 2 == 1)
            rws = rows[sq * 128:(sq + 1) * 128]
            wb = (w_in[rws, :] * (1.0 / math.sqrt(HD))) if is_q else w_in[E + rws, :]
            # wb [128 m, 1024 e] -> [p, ec, m]
            wqk[:, s] = wb.T.reshape(8, 128, 128).transpose(1, 0, 2).astype(
                ml_dtypes.bfloat16)

        wv = w_in[2 * E + rows, :]                               # [256, E]
        wvT_ = np.ascontiguousarray(
            wv.T.reshape(8, 128, 256).transpose(1, 0, 2)).astype(ml_dtypes.bfloat16)

        xTa = np.ascontiguousarray(
            q32[b].T.reshape(8, 128, T).transpose(1, 0, 2)).astype(ml_dtypes.bfloat16)

        wo_loc = np.ascontiguousarray(
            w_out[:, rows].T.reshape(2, 128, E).transpose(1, 0, 2)).astype(
                ml_dtypes.bfloat16)

        biasd = np.empty((128, HL, 640), ml_dtypes.bfloat16)
        for hl in range(HL):
            s = slopes[4 * hq + hl]
            eb = np.where(allowed, np.exp(-s * (cc - jj).astype(np.float64)), 0.0)
            biasd[:, hl] = eb.astype(ml_dtypes.bfloat16)

        in_maps.append(
            {"xT": xTa, "wqk": wqk, "wvT": wvT_, "wo": wo_loc, "biasd": biasd})
    return in_maps


def _assemble(results):
    out = np.zeros((B, T, E), np.float32)
    for c in range(NCORES):
        b = c // 4
        part = np.asarray(results[c]["outT"]).astype(np.float32)  # [8,128,T]
        out[b] += part.reshape(E, T).T
    return out


def kernel(query, in_proj_weight, out_proj_weight, num_heads, sliding_window_size):
    assert int(num_heads) == H and int(sliding_window_size) == W
    assert query.shape == (B, T, E)
    if "nc" not in _CACHE:
        _CACHE["nc"] = _build()
    in_maps = _host_inputs(query, in_proj_weight, out_proj_weight)
    res = run_bass_kernel_spmd(_CACHE["nc"], in_maps, list(range(NCORES))).results
    return _assemble(res)
